# revision 36
# baseline (speedup 1.0000x reference)
"""DualAttention Trainium2 kernel (v3: fp8 DoubleRow scores + woven schedule).

Sharding: 8 cores = 4 samples x 2 query-halves. Per core the sample image is
"rolled" by the half offset (host-side, with correct zero padding), so every
core runs the identical program on its first 2048 query positions; attention
over key positions is permutation-invariant, so convs/attention on the rolled
image give the true result for the core's half.

Precision plan (validated vs reference in numpy, rel err ~1.2e-2):
  qk conv: bf16 image x bf16 weights (w pre-scaled x16), f32 PSUM.
  scores: q,k quantized to fp8(16q) by the conv epilogue, pair-packed
    [32,2,pos] (c -> partition c%32, block c//32), fp8 DoubleRow matmul;
    1/sqrt(Cr)/256 folded into the exp scale.
  v conv: fp8e4m3 DoubleRow (x and 64*wv in fp8), batched multi-tile xbar
    DMA transpose (bf16) -> fp8 vt8 (8*v).
  exp -> fp8 E tiles (exp(s/2048 - 3)).
  U = vt8 @ E8 and denominator = ones8 @ E8, fp8 DoubleRow, 512-wide
    PSUM-bank-aligned chunks.
  loc staged bf16; recb bf16; glob bf16; fuse bf16xbf16 / bf16 matmuls.

Schedule: conv chunks, SE, glob, and v-conv quanta are woven into the h0
score/exp loop; U(h0), denom(h0), recip/loc, fuse(h0)+out(h0), and the
leading pairs of U(h1) are woven into the h1 (gen1) loop; only denom(h1),
the last U pairs, and fuse(h1) trail the final exp.
"""

import sys

sys.path.insert(0, "/opt/trn_rl_repo")

import numpy as np
import ml_dtypes

import concourse.bass as bass
import concourse.mybir as mybir
import concourse.tile as tile
from concourse import bacc
from concourse.bass_utils import run_bass_kernel_spmd

f32 = mybir.dt.float32
f32r = mybir.dt.float32r
fp8 = mybir.dt.float8e4
bf16 = mybir.dt.bfloat16
AF = mybir.ActivationFunctionType
DR = mybir.MatmulPerfMode.DoubleRow
e4m3 = ml_dtypes.float8_e4m3fn

C = 256
CT = 2          # channel tiles of 128
Cr = 64
H = W = 64
HW = H * W      # 4096
HWh = 2048      # query positions per core
JT = 32         # key-position tiles of 128
IH = 2          # i halves of 1024
ICH = 2         # 512-chunks per i half
N_CORES = 8
EXPB = 3.0      # exp bias: E = exp(s - EXPB), cancels in softmax
QKS = 16.0      # q/k stored as fp8(16*q), fp8(16*k)
EXP_SCALE = float(Cr ** -0.5) / (QKS * QKS)   # folds 1/sqrt(Cr) and the 16x16

_compiled = None


def _build(debug=False, parts=("qk", "vt8", "loc", "glob", "et", "recb", "usb")):
    nc = bacc.Bacc("TRN2", target_bir_lowering=False, debug=False,
                   num_devices=N_CORES)
    parts = set(parts) if debug else set()

    xp_d = nc.declare_dram_parameter("xp", [C, 68 * 66], bf16, isOutput=False)
    xp8_d = nc.declare_dram_parameter("xp8", [128, 2 * 68 * 66], fp8, isOutput=False)
    wqkt_d = nc.declare_dram_parameter("wqkt", [18, 128, 128], bf16, isOutput=False)
    wv8_d = nc.declare_dram_parameter("wv8", [9, 128, 512], fp8, isOutput=False)
    smallp_d = nc.declare_dram_parameter("smallp", [128, 37], f32, isOutput=False)
    smallq_d = nc.declare_dram_parameter("smallq", [16, 257], f32, isOutput=False)
    fuset_d = nc.declare_dram_parameter("fuset", [4, 2, 128, 128], bf16, isOutput=False)
    out_d = nc.declare_dram_parameter("out", [2, 128, HWh], f32, isOutput=True)
    if "qkpin" in parts:
        qk_dbg = nc.declare_dram_parameter("qk_dbg", [128, 64], f32, isOutput=True)
    elif "qk" in parts:
        qk_dbg = nc.declare_dram_parameter("qk_dbg", [128, HW], f32, isOutput=True)
    if "vt8" in parts:
        vt8_dbg = nc.declare_dram_parameter("vt8_dbg", [128, 16 * 512], fp8, isOutput=True)
    if "loc" in parts:
        loc_dbg = nc.declare_dram_parameter("loc_dbg", [2, 128, HWh], f32, isOutput=True)
    if "et" in parts:
        et_dbg = nc.declare_dram_parameter("et_dbg", [16, 128, 2048], fp8, isOutput=True)
    if "recb" in parts:
        recb_dbg = nc.declare_dram_parameter("recb_dbg", [2, 128, 1024], f32, isOutput=True)
    if "usb" in parts:
        usb_dbg = nc.declare_dram_parameter("usb_dbg", [2, 128, HWh], f32, isOutput=True)
    if "glob" in parts:
        glob_dbg = nc.declare_dram_parameter("glob_dbg", [2, 128, HWh], f32, isOutput=True)

    with tile.TileContext(nc) as tc, \
         nc.allow_low_precision(reason="fp8/f32r storage; validated numerics"):
      with tc.tile_pool(name="pw", bufs=1) as pw, \
           tc.tile_pool(name="pxv", bufs=1) as pxv:
        # persistent tiles
        wqkt = pw.tile([128, 18, 128], bf16)
        w8v = pw.tile([128, 9, 2, 256], fp8)
        fuset = pw.tile([128, 8, 128], bf16)
        sp = pw.tile([128, 37], f32)
        sq = pw.tile([16, 257], f32)

        qk8 = pw.tile([128, HW], fp8)         # q rows 0-63 (cols 0-2047), k rows 64-127
        # DR-packed contract pairs, channel c -> (partition c%32, j block c//32);
        # separate tiles so stationary/moving share base partition 0
        k8 = pw.tile([32, 2, HW], fp8)
        q8 = pw.tile([32, 2, HWh], fp8)
        vt8 = pw.tile([128, 16, 2, 256], fp8)  # v^T, fp8(8*v), pair-slotted
        glob = [pw.tile([128, 32, 64], bf16, tag=f"glob{t}", name=f"glob{t}") for t in range(CT)]
        yse = [pw.tile([128, 1], f32, tag=f"yse{t}", name=f"yse{t}") for t in range(CT)]
        loc = [pw.tile([128, HWh], bf16, tag=f"loc{t}", name=f"loc{t}") for t in range(CT)]
        recb = pw.tile([128, 1024], bf16)
        ones8 = pw.tile([128, 2, 128], fp8)
        bneg = pw.tile([128, 1], f32)
        warm = pw.tile([128, 1], f32)

        # pet/psT span both the conv phase (h0 loop is woven into the conv)
        # and the gen1/U phase.
        with tc.tile_pool(name="pet", bufs=26) as pet, \
             tc.tile_pool(name="psT", bufs=2, space="PSUM") as psT:
          et_halves = {0: [], 1: []}

          with tc.tile_pool(name="px", bufs=1) as px, \
               tc.tile_pool(name="psqk", bufs=2, space="PSUM") as psqk, \
               tc.tile_pool(name="psv", bufs=1, space="PSUM") as psv, \
               tc.tile_pool(name="psse", bufs=1, space="PSUM") as psse:
            xp = [px.tile([128, 68, 66], bf16, tag=f"xp{j}", name=f"xp{j}") for j in range(CT)]
            xp8 = pxv.tile([128, 2, 68, 66], fp8)
            v_sb = [pxv.tile([128, HW], bf16, tag=f"vsb{t}", name=f"vsb{t}")
                    for t in range(CT)]
            vt_b = pxv.tile([128, 32, 256], bf16)

            xsrc = [xp_d[j * 128:(j + 1) * 128, :].rearrange(
                "p (h w) -> p h w", w=66) for j in range(CT)]
            x8src = xp8_d[:].rearrange("p (a b c) -> p a b c", b=68, c=66)
            # head DMAs: only what the four head conv chunks need (rows
            # 0:23 + weights); the k8/q8 copies are emitted right after the
            # head chunks so their descriptors are not stuck behind the bulk
            # input load, and split across the sync+gpsimd queues
            nc.sync.dma_start(wqkt[:, 0:6:2, :],
                              wqkt_d[0:6:2].rearrange("t p m -> p t m"))
            for j in range(CT):
                nc.sync.dma_start(xp[j][:, 0:6, :], xsrc[j][:, 0:6, :])
            nc.sync.dma_start(wqkt[:, 6:18:2, :],
                              wqkt_d[6:18:2].rearrange("t p m -> p t m"))
            nc.sync.dma_start(wqkt[:, 1:18:2, :],
                              wqkt_d[1:18:2].rearrange("t p m -> p t m"))
            nc.sync.dma_start(sp[:], smallp_d[:])
            for j in range(CT):
                nc.sync.dma_start(xp[j][:, 6:12, :], xsrc[j][:, 6:12, :])
            nc.sync.dma_start(
                w8v[:].rearrange("p t a b -> p t (a b)"),
                wv8_d[:].rearrange("t p m -> p t m"))
            nc.sync.dma_start(xp8[:, :, 0:34, :], x8src[:, :, 0:34, :])
            for j in range(CT):
                nc.sync.dma_start(xp[j][:, 12:23, :], xsrc[j][:, 12:23, :])
            nc.sync.dma_start(sq[:], smallq_d[:])
            nc.sync.dma_start(xp8[:, :, 34:68, :], x8src[:, :, 34:68, :])
            for r0, r1 in [(23, 34), (34, 46), (46, 57), (57, 68)]:
                for j in range(CT):
                    nc.sync.dma_start(xp[j][:, r0:r1, :], xsrc[j][:, r0:r1, :])
            nc.sync.dma_start(
                fuset[:].rearrange("p (k m) f -> p k m f", k=4),
                fuset_d[:].rearrange("k m p f -> p k m f"),
            )

            # constants; the dummy exp pulls the ACT table load off the
            # first-score critical path
            nc.vector.memset(ones8[:], 1.0)
            nc.vector.memset(bneg[:], -EXPB)
            nc.scalar.activation(warm[:], bneg[:], AF.Exp)

            # ---- fused q+k conv chunks (co=128, fp32r): ALL 4-row chunks
            # (1.9us atomic chains) so the woven PE stream never starves the
            # exp pipeline and never stalls on psT buffers
            qk_bases = [0, 4, 8, 12, 16, 20, 24, 28,
                        34, 38, 42, 46, 50, 54, 58, 62]

            def emit_chunk(base):
                nrow = 4
                pqk = psqk.tile([128, 256], f32, name="pqk")
                first = True
                for j in range(CT):
                    for dy in range(3):
                        for dx in range(3):
                            t = (dy * 3 + dx) * 2 + j
                            nc.tensor.matmul(
                                pqk[:],
                                wqkt[:, t, :],
                                xp[j][:, base + dy:base + dy + nrow, dx:dx + 64],
                                start=first,
                                stop=(t == 17),
                            )
                            first = False
                o0 = (base if base < 34 else base - 2) * 64
                nc.vector.tensor_scalar_add(
                    qk8[:, o0:o0 + nrow * 64], pqk[:], sp[:, 0:1])

            def emit_kq_copy(p0, p1, with_q, eng=None):
                # batched pair-packing copies (swdge fixed cost ~1us per dma,
                # so copy wide spans, not per-chunk slivers)
                eng = eng or nc.gpsimd
                csl = slice(p0, p1)
                if with_q:
                    eng.dma_start(q8[:, 0, csl], qk8[0:32, csl])
                    eng.dma_start(q8[:, 1, csl], qk8[32:64, csl])
                eng.dma_start(k8[:, 0, csl], qk8[64:96, csl])
                eng.dma_start(k8[:, 1, csl], qk8[96:128, csl])

            def emit_se():
                # ---- SE channel sums (mean folded into fc1 host-side)
                sums = [pw.tile([128, 1], f32, tag=f"sums{j}", name=f"sums{j}")
                        for j in range(CT)]
                sa = pw.tile([128, 1], f32)
                sb_ = pw.tile([128, 1], f32)
                for j in range(CT):
                    nc.vector.reduce_sum(sa[:], xp[j][:, 1:33, 1:65],
                                         axis=mybir.AxisListType.XY)
                    nc.vector.reduce_sum(sb_[:], xp[j][:, 35:67, 1:65],
                                         axis=mybir.AxisListType.XY)
                    nc.vector.tensor_add(sums[j][:], sa[:], sb_[:])
                # ---- SE MLP: y = sigmoid(fc2 @ relu(fc1 @ mean + b1) + b2)
                ps1 = psse.tile([16, 1], f32)
                for j in range(CT):
                    nc.tensor.matmul(ps1[:], sp[:, 5 + j * 16:5 + (j + 1) * 16],
                                     sums[j][:],
                                     start=(j == 0), stop=(j == CT - 1))
                y1 = pw.tile([16, 1], f32)
                nc.scalar.activation(y1[:], ps1[:], AF.Relu, bias=sq[0:16, 256:257])
                for t in range(CT):
                    ps2 = psse.tile([128, 1], f32, tag="ps1", name="ps2")
                    nc.tensor.matmul(ps2[:], sq[0:16, t * 128:(t + 1) * 128], y1[:],
                                     start=True, stop=True)
                    # sigmoid(z) = 1/(1+exp(-z)), z = ps2 + fc2b ; fc2bn = -fc2b
                    en = pw.tile([128, 1], f32, tag="en")
                    nc.scalar.activation(en[:], ps2[:], AF.Exp,
                                         bias=sp[:, 3 + t:4 + t], scale=-1.0)
                    nc.vector.tensor_scalar_add(en[:], en[:], 1.0)
                    nc.vector.reciprocal(yse[t][:], en[:])

            def emit_glob():
                # ---- glob = x_half * yse (before xp pool closes)
                for t in range(CT):
                    nc.vector.tensor_scalar_mul(glob[t][:],
                                                xp[t][:, 1:33, 1:65],
                                                yse[t][:, 0:1])

            def emit_vq(qi):
                # one v-conv quantum + batched transpose/staging every 4
                ct, ch = qi // 16, qi % 16
                base = ch * 4 if ch < 8 else 34 + (ch - 8) * 4
                pv = psv.tile([128, 256], f32, tag="pv", name="pv")
                for dy in range(3):
                    for dx in range(3):
                        t = dy * 3 + dx
                        nc.tensor.matmul(
                            pv[:],
                            w8v[:, t, :, ct * 128:(ct + 1) * 128],
                            xp8[:, :, base + dy:base + dy + 4, dx:dx + 64],
                            start=(t == 0), stop=(t == 8), perf_mode=DR)
                nc.vector.tensor_scalar_mul(
                    v_sb[ct][:, ch * 256:(ch + 1) * 256], pv[:], 0.125)
                if ch % 4 == 3:
                    # batched multi-tile xbar transpose: 8 column blocks in
                    # ONE dma (hwdge fixed cost paid once)
                    c0 = ch - 3
                    nc.sync.dma_start_transpose(
                        vt_b[:, 2 * c0:2 * c0 + 8, ct * 128:(ct + 1) * 128],
                        v_sb[ct][:, c0 * 256:(c0 + 4) * 256])
                    if ct == 1:
                        # both ct chunks of pairs c0..c0+3 done: stage fp8
                        nc.vector.tensor_copy(
                            vt8[:, c0:c0 + 4, :, :],
                            vt_b[:, 2 * c0:2 * c0 + 8, :])

            def emit_A(ih, weave, et_half):
                # scores+exp steps; during half 0 the remaining conv chunks,
                # SE, glob, and the v-conv quanta are woven into the jt steps
                # (fills the ACT-bound gaps on PE)
                i0 = ih * 1024
                for jt in range(JT):
                    pT = psT.tile([128, 1024], f32, tag="pT", name="pT")
                    for icq in range(ICH):
                        isl = slice(i0 + icq * 512, i0 + (icq + 1) * 512)
                        psl = pT[:, icq * 512:(icq + 1) * 512]
                        nc.tensor.matmul(psl,
                                         k8[:, :, jt * 128:(jt + 1) * 128],
                                         q8[:, :, isl],
                                         start=True, stop=True, perf_mode=DR)
                    if jt % 2 == 0:
                        et = pet.tile([128, 2, 1024], fp8, tag="et", name="et")
                        et_half.append(et)
                    nc.scalar.activation(et[:, jt % 2, :], pT[:], AF.Exp,
                                         bias=bneg[:, 0:1], scale=EXP_SCALE)
                    if weave and jt >= 4:
                        emit_vq(jt - 4)
                    yield jt

            # h0 attention loop woven into the conv: the first 4 chunks
            # cover q half 0 (pos 0:1024) and k tiles jt 0-7; the remaining
            # 12 interleave one per jt step
            for base in qk_bases[:4]:
                emit_chunk(base)
            emit_kq_copy(0, 1024, True, eng=nc.sync)
            for jt in emit_A(0, True, et_halves[0]):
                if jt < 12:
                    emit_chunk(qk_bases[4 + jt])
                    if jt == 3:
                        emit_kq_copy(1024, 2048, True)
                    elif jt == 7:
                        emit_kq_copy(2048, 3072, False)
                    elif jt == 11:
                        emit_kq_copy(3072, 4096, False)
                elif jt == 13:
                    emit_se()
                elif jt == 15:
                    emit_glob()
            for qi in range(28, 32):
                emit_vq(qi)

          with tc.tile_pool(name="psU", bufs=1, space="PSUM") as psU, \
               tc.tile_pool(name="po", bufs=2) as po:
            # gen1 (half-1 scores+exp) paces ACT; U/denom/fuse work for half 0
            # and the leading pairs of half 1 are woven into its steps so PE
            # stays busy. PSUM tags pu0/pu1 cycle: pu(h0) -> pDB(h0) /
            # fuse-psum -> pu(h1) -> pDB(h1) / fuse-psum.
            gen1 = emit_A(1, None, et_halves[1])
            et0, et1 = et_halves[0], et_halves[1]

            def step():
                try:
                    next(gen1)
                except StopIteration:
                    pass

            def emit_useg(pu, et_half, t, p):
                for icq in range(2):
                    nc.tensor.matmul(
                        pu[t][:, icq * 512:(icq + 1) * 512],
                        vt8[:, p, :, t * 128:(t + 1) * 128],
                        et_half[p][:, :, icq * 512:(icq + 1) * 512],
                        start=(p == 0), stop=(p == 15),
                        perf_mode=DR, skip_group_check=True)

            def emit_dseg(pDB, et_half, p):
                for icq in range(2):
                    nc.tensor.matmul(
                        pDB[:, icq * 512:(icq + 1) * 512],
                        ones8[:],
                        et_half[p][:, :, icq * 512:(icq + 1) * 512],
                        start=(p == 0), stop=(p == 15),
                        perf_mode=DR, skip_group_check=True)

            def emit_loc_stage(pu, i0, act):
                # stage U into loc (tensor_copy/activation reads of PSUM are
                # ordered correctly vs later bank reuse; direct tensor_tensor
                # reads of PSUM race on hardware); ACT copy only where ACT
                # is not the pacing engine
                nc.vector.tensor_copy(loc[0][:, i0:i0 + 1024], pu[0][:])
                if act:
                    nc.scalar.activation(loc[1][:, i0:i0 + 1024], pu[1][:],
                                         AF.Copy)
                else:
                    nc.vector.tensor_copy(loc[1][:, i0:i0 + 1024], pu[1][:])

            def emit_loc_mul(i0, icq):
                isl = slice(i0 + icq * 512, i0 + (icq + 1) * 512)
                rsl = slice(icq * 512, (icq + 1) * 512)
                for t in range(CT):
                    nc.vector.tensor_mul(loc[t][:, isl], loc[t][:, isl],
                                         recb[:, rsl])

            def emit_fuse_chain(pfm, mt, icq):
                # fuse 1x1 conv (bv and fuse_b folded host-side)
                pf = pfm[:, (icq % 2) * 512:(icq % 2 + 1) * 512]
                isl = slice(icq * 512, (icq + 1) * 512)
                for kt in range(4):
                    rhs = (loc[kt][:, isl] if kt < 2 else
                           glob[kt - 2][:, icq * 8:(icq + 1) * 8, :])
                    nc.tensor.matmul(pf, fuset[:, kt * 2 + mt, :],
                                     rhs, start=(kt == 0), stop=(kt == 3))

            def emit_fuse_out(pfm, ob, mt, icq, act):
                sub = icq % 2
                pf = pfm[:, sub * 512:(sub + 1) * 512]
                if act:
                    nc.scalar.activation(ob[:, sub * 512:(sub + 1) * 512], pf,
                                         AF.Identity, bias=sp[:, 1 + mt:2 + mt])
                else:
                    nc.vector.tensor_scalar_add(
                        ob[:, sub * 512:(sub + 1) * 512], pf,
                        sp[:, 1 + mt:2 + mt])
                nc.sync.dma_start(out_d[mt, :, icq * 512:(icq + 1) * 512],
                                  ob[:, sub * 512:(sub + 1) * 512])

            # --- woven schedule over gen1's 32 steps ---
            pu = [psU.tile([128, 1024], f32, tag=f"pu{t}", name=f"pu{t}")
                  for t in range(CT)]
            segs0 = [(t, p) for t in range(CT) for p in range(16)]
            si = 0
            for s in range(14):               # U(h0): 32 segs over 14 steps
                step()
                want = (s + 1) * 32 // 14
                while si < want:
                    emit_useg(pu, et0, *segs0[si])
                    si += 1
            emit_loc_stage(pu, 0, act=False)
            pDB0 = psU.tile([128, 1024], f32, tag="pu0", name="pDB0")
            for s in range(14, 18):           # denom(h0): 16 p-segs over 4
                step()
                for p in range(4 * (s - 14), 4 * (s - 13)):
                    emit_dseg(pDB0, et0, p)
            step()
            nc.vector.reciprocal(recb[:, 0:512], pDB0[:, 0:512])
            emit_loc_mul(0, 0)
            step()
            nc.vector.reciprocal(recb[:, 512:1024], pDB0[:, 512:1024])
            step()
            emit_loc_mul(0, 1)
            pfm0 = psU.tile([128, 1024], f32, tag="pu1", name="pfm0")
            ob0 = po.tile([128, 1024], f32, tag="ob", name="ob0")
            step()
            emit_fuse_chain(pfm0, 0, 0)
            emit_fuse_chain(pfm0, 0, 1)
            emit_fuse_out(pfm0, ob0, 0, 0, act=False)
            step()
            emit_fuse_out(pfm0, ob0, 0, 1, act=False)
            pfm1 = psU.tile([128, 1024], f32, tag="pu0", name="pfm1")
            ob1 = po.tile([128, 1024], f32, tag="ob", name="ob1")
            step()
            emit_fuse_chain(pfm1, 1, 0)
            emit_fuse_chain(pfm1, 1, 1)
            emit_fuse_out(pfm1, ob1, 1, 0, act=False)
            emit_fuse_out(pfm1, ob1, 1, 1, act=False)
            # trailing U(h1) + late denom(h1) woven into the last steps
            pu = [psU.tile([128, 1024], f32, tag=f"pu{t}", name=f"pu{t}")
                  for t in range(CT)]
            h1p = 0
            d1p = 0
            for s in range(24, 32):
                step()
                while h1p <= (s - 3) // 2 and h1p < 16:
                    for t in range(CT):
                        emit_useg(pu, et1, t, h1p)
                    h1p += 1
            while h1p < 16:
                for t in range(CT):
                    emit_useg(pu, et1, t, h1p)
                h1p += 1
            emit_loc_stage(pu, 1024, act=True)
            pDB1 = psT.tile([128, 1024], f32, tag="pT", name="pDB1")
            while d1p < 16:
                emit_dseg(pDB1, et1, d1p)
                d1p += 1
            nc.vector.reciprocal(recb[:, 0:512], pDB1[:, 0:512])
            nc.vector.reciprocal(recb[:, 512:1024], pDB1[:, 512:1024])
            # 512-granular loc-mul -> fuse -> out pipeline for the tail
            pfm0 = psU.tile([128, 1024], f32, tag="pu1", name="pfm0b")
            pfm1 = psU.tile([128, 1024], f32, tag="pu0", name="pfm1b")
            ob0 = po.tile([128, 1024], f32, tag="ob", name="ob0b")
            ob1 = po.tile([128, 1024], f32, tag="ob", name="ob1b")
            for icq in (2, 3):
                emit_loc_mul(1024, icq - 2)
                emit_fuse_chain(pfm0, 0, icq)
                emit_fuse_chain(pfm1, 1, icq)
                emit_fuse_out(pfm0, ob0, 0, icq, act=False)
                emit_fuse_out(pfm1, ob1, 1, icq, act=True)

          with tc.tile_pool(name="pdbg", bufs=1) as pdbg:
            if "qkpin" in parts:
                nc.sync.dma_start(qk_dbg[:], qk8[:, 0:256].bitcast(f32))
            elif "qk" in parts:
                nc.sync.dma_start(qk_dbg[:, 0:1024], qk8[:].bitcast(f32))
            if "vt8" in parts:
                nc.sync.dma_start(
                    vt8_dbg[:],
                    vt8[:].rearrange("p a b c -> p (a b c)"))
            for t in range(CT):
                if "loc" in parts:
                    nc.sync.dma_start(loc_dbg[t], loc[t][:].bitcast(f32))
                if "glob" in parts:
                    nc.sync.dma_start(
                        glob_dbg[t],
                        glob[t][:].rearrange("p a b -> p (a b)").bitcast(f32))

    nc.compile()
    return nc


def _prep_core_inputs(inputs):
    x = np.ascontiguousarray(inputs["x"], np.float32)
    wq = np.asarray(inputs["wq"], np.float32)
    bq = np.asarray(inputs["bq"], np.float32)
    wk = np.asarray(inputs["wk"], np.float32)
    bk = np.asarray(inputs["bk"], np.float32)
    wv = np.asarray(inputs["wv"], np.float32)
    bv = np.asarray(inputs["bv"], np.float32)
    fc1_w = np.asarray(inputs["fc1_w"], np.float32)
    fc1_b = np.asarray(inputs["fc1_b"], np.float32)
    fc2_w = np.asarray(inputs["fc2_w"], np.float32)
    fc2_b = np.asarray(inputs["fc2_b"], np.float32)
    fuse_w = np.asarray(inputs["fuse_w"], np.float32)[:, :, 0, 0]
    fuse_b = np.asarray(inputs["fuse_b"], np.float32)

    # q,k stored as 16*q, 16*k in fp8; 1/sqrt(Cr)/256 folded into the exp scale
    wqk = np.concatenate([wq * QKS, wk * QKS], axis=0)      # [128, 256, 3, 3]
    bqk = np.concatenate([bq * QKS, bk * QKS])[:, None].astype(np.float32)

    wqkt = np.empty((18, 128, 128), np.float32)
    for dy in range(3):
        for dx in range(3):
            for j in range(CT):
                t = (dy * 3 + dx) * 2 + j
                wqkt[t] = wqk[:, j * 128:(j + 1) * 128, dy, dx].T

    # fp8 v weights (x64), layout [tap, ci_lo(128), ci_tile(2)*co(256)]
    wv8 = np.empty((9, 128, 512), np.float32)
    for dy in range(3):
        for dx in range(3):
            t = dy * 3 + dx
            for tci in range(CT):
                # [co, ci128] -> [ci128, co]
                wv8[t, :, tci * 256:(tci + 1) * 256] = \
                    (64.0 * wv[:, tci * 128:(tci + 1) * 128, dy, dx]).T
    wv8 = np.clip(wv8, -448, 448).astype(e4m3)

    # fuse: local half carries 1/8 (v stored as 8*v)
    fuse_b_eff = fuse_b + fuse_w[:, :C] @ bv
    fuset = np.empty((4, 2, 128, 128), np.float32)
    for kt in range(4):
        s = 0.125 if kt < 2 else 1.0
        for mt in range(CT):
            fuset[kt, mt] = (s * fuse_w[mt * 128:(mt + 1) * 128,
                                        kt * 128:(kt + 1) * 128]).T

    smallp = np.zeros((128, 37), np.float32)
    smallp[:, 0:1] = bqk
    smallp[:, 1:3] = np.stack([fuse_b_eff[t * 128:(t + 1) * 128] for t in range(CT)], axis=1)
    smallp[:, 3:5] = np.stack([-fc2_b[t * 128:(t + 1) * 128] for t in range(CT)], axis=1)
    for j in range(CT):
        smallp[:, 5 + j * 16:5 + (j + 1) * 16] = (fc1_w / HW)[:, j * 128:(j + 1) * 128].T
    smallq = np.zeros((16, 257), np.float32)
    for t in range(CT):
        smallq[:, t * 128:(t + 1) * 128] = fc2_w[t * 128:(t + 1) * 128, :].T
    smallq[:, 256] = fc1_b
    shared = dict(
        wqkt=np.ascontiguousarray(wqkt.astype(ml_dtypes.bfloat16)).view(np.uint16),
        wv8=wv8.view(np.uint8),
        fuset=np.ascontiguousarray(fuset.astype(ml_dtypes.bfloat16)).view(np.uint16),
        smallp=smallp, smallq=smallq,
    )

    in_maps = []
    for core in range(N_CORES):
        s, p = divmod(core, 2)
        s0 = p * 32
        t0 = (s0 + 32) % 64
        P = np.zeros((C, 66, 66), np.float32)
        P[:, 1:65, 1:65] = x[s]
        xp = np.concatenate([P[:, s0:s0 + 34], P[:, t0:t0 + 34]], axis=1)
        m = dict(shared)
        m["xp"] = np.ascontiguousarray(
            xp.reshape(C, 68 * 66).astype(ml_dtypes.bfloat16)).view(np.uint16)
        xp8 = np.clip(xp, -448, 448).astype(e4m3)  # [256, 68, 66]
        m["xp8"] = np.ascontiguousarray(
            xp8.reshape(2, 128, 68 * 66).transpose(1, 0, 2).reshape(128, -1)
        ).view(np.uint8)
        in_maps.append(m)
    return in_maps


def kernel(**inputs):
    global _compiled
    if _compiled is None:
        # parts=("qkpin",) keeps one trailing debug DMA of a qk slice: it
        # pins qk's liveness to the end of the program, which shifts tile
        # buffer assignment such that the schedule is correct on hardware
        # (without it, a buffer-reuse race corrupts the local-attention
        # accumulator).
        _compiled = _build(debug=True, parts=("qkpin",))
    nc = _compiled
    in_maps = _prep_core_inputs(inputs)
    res = run_bass_kernel_spmd(nc, in_maps, list(range(N_CORES)))
    out = np.empty((4, C, H, W), np.float32)
    for core in range(N_CORES):
        s, p = divmod(core, 2)
        o = res.results[core]["out"]          # [2, 128, 2048]
        out[s, :, p * 32:(p + 1) * 32, :] = o.reshape(C, 32, 64)
    return out

